# revision 1
# baseline (speedup 1.0000x reference)
"""GAT discriminator (gnn_message_passing) on 8 Trainium2 NeuronCores.

Strategy (sharding hint: partition nodes/edges, replicate small weights,
all-reduce pooled sums):
  - Nodes are partitioned into 8 contiguous ranges; each core owns the edges
    whose dst falls in its range (host-side graph partitioning, index math
    only -- no float compute on host).
  - Phase A (device): each core projects its own nodes
    featAug[n] = [feat(192) | el(3)] and er[n] (3), then two AllGathers
    replicate the full node tables to every core's DRAM.
  - Phase B (device, per 2048-edge tile): one indirect-DMA gather of
    featAug[src] rows, one 12B/edge gather of er[dst]; edge-major vector math
    w = exp(leaky_relu(el+er)); segment softmax + aggregation via one-hot
    matmuls (segments are contiguous runs of dst, never crossing tiles);
    per-graph pooling folded into a second tiny matmul, accumulated in SBUF.
  - Finish: AllReduce of pooled [16,64] partial sums, divide by per-graph
    node counts, tiny MLP + sigmoid on-device, output [1,16].

The edge softmax skips the segment-max subtraction: e = leaky_relu(el+er) is
bounded (|e| < ~10), so exp(e) is well within f32 range and
alpha = exp(e)/sum(exp(e)) is mathematically identical to the max-subtracted
form used by the reference.
"""

import numpy as np
from contextlib import ExitStack

import concourse.bass as bass
import concourse.bacc as bacc
import concourse.mybir as mybir
import concourse.tile as tile
from concourse.bass_utils import run_bass_kernel_spmd

F32 = mybir.dt.float32
F32R = mybir.dt.float32r
I32 = mybir.dt.int32

# Problem dims (overridable for small-scale sim tests via configure()).
N, E, G = 50000, 800000, 16
NODE, HID, OUT, HEADS = 128, 64, 128, 3
NEG_SLOPE = 0.2
NCORES = 8

TEDGE = 2048          # edge slots per tile
KCH = TEDGE // 128    # chunks per tile
SMAX = 128            # segment slots per tile (row SMAX-1 reserved for pads)
FW = HEADS * HID      # 192 feat width
AUGW = FW + HEADS     # 195: feat + el
WAUGW = FW + 2 * HEADS  # 198: feat + el + er
XW = 256              # X free width padded for f32r full-rate matmul
MM_FAST = True        # True: float32r matmuls (fast, rounded); False: exact f32
DEBUG_TAPS = False    # extra debug output params
DBG_TILE = 3          # which tile index to tap


def configure(n, e, g, mm_fast=None, debug_taps=None):
    global N, E, G, MM_FAST, DEBUG_TAPS
    N, E, G = n, e, g
    if mm_fast is not None:
        MM_FAST = mm_fast
    if debug_taps is not None:
        DEBUG_TAPS = debug_taps


# ---------------------------------------------------------------------------
# Host-side graph partitioning (pure index math)
# ---------------------------------------------------------------------------

def _pack_tiles(e_src, e_dst, e_seg_gid, graph_of_node, own_lo):
    """Pack dst-sorted edges of one core into tiles.

    Returns list of per-tile dicts with fixed-size arrays.
    """
    n_e = e_src.shape[0]
    tiles = []
    # segment boundaries over sorted dst
    if n_e == 0:
        return tiles
    uniq, first_idx, counts = np.unique(
        e_dst, return_index=True, return_counts=True)
    n_seg = uniq.shape[0]
    s = 0
    while s < n_seg:
        e_start = first_idx[s]
        n_edges = 0
        n_segs = 0
        while (s + n_segs < n_seg and n_segs < SMAX - 1
               and n_edges + counts[s + n_segs] <= TEDGE):
            n_edges += counts[s + n_segs]
            n_segs += 1
        assert n_segs > 0, "single segment larger than TEDGE"
        seg_ids = uniq[s:s + n_segs]            # global node ids of segments
        src_t = e_src[e_start:e_start + n_edges]
        dst_t = e_dst[e_start:e_start + n_edges]
        # local segment index per edge
        seg_local = np.searchsorted(seg_ids, dst_t).astype(np.int32)
        # pad to TEDGE
        pad = TEDGE - n_edges
        src_p = np.concatenate([src_t, np.zeros(pad, np.int32)])
        dst_p = np.concatenate([dst_t, np.zeros(pad, np.int32)])
        seg_p = np.concatenate(
            [seg_local, np.full(pad, SMAX - 1, np.int32)])
        gm = np.zeros((SMAX, G), np.float32)
        gm[np.arange(n_segs), graph_of_node[seg_ids]] = 1.0 / 3.0
        tiles.append(dict(src=src_p, dst=dst_p, seg=seg_p, gm=gm))
        s += n_segs
    return tiles


def _host_prepare(inputs):
    """Build all per-core staged arrays from the full inputs."""
    src = np.asarray(inputs["src"], np.int32)
    dst = np.asarray(inputs["dst"], np.int32)
    graph_id = np.asarray(inputs["graph_id"], np.int32)
    h = np.asarray(inputs["h"], np.float32)
    z = np.asarray(inputs["z"], np.float32)
    fc_w = np.asarray(inputs["fc_w"], np.float32)
    attn_l = np.asarray(inputs["attn_l"], np.float32)
    attn_r = np.asarray(inputs["attn_r"], np.float32)
    bias = np.asarray(inputs["bias"], np.float32)
    lin1_w = np.asarray(inputs["lin1_w"], np.float32)
    lin1_b = np.asarray(inputs["lin1_b"], np.float32)
    lin2_w = np.asarray(inputs["lin2_w"], np.float32)
    lin2_b = np.asarray(inputs["lin2_b"], np.float32)

    own = N // NCORES
    assert own * NCORES == N

    # per-core edge lists, dst-sorted
    core_of = dst // own
    order = np.lexsort((dst,))
    src_s, dst_s, core_s = src[order], dst[order], core_of[order]

    per_core_tiles = []
    for c in range(NCORES):
        m = core_s == c
        per_core_tiles.append(
            _pack_tiles(src_s[m], dst_s[m], None, graph_id, c * own))
    n_tiles = max(max((len(t) for t in per_core_tiles)), 1)

    # weight folding (weight-only preprocessing)
    fc3 = fc_w.reshape(HEADS, HID, NODE)
    w_l = np.einsum("kdf,kd->fk", fc3, attn_l).astype(np.float32)  # [NODE,3]
    w_r = np.einsum("kdf,kd->fk", fc3, attn_r).astype(np.float32)
    waugT = np.concatenate([fc_w.T, w_l, w_r], axis=1)  # [NODE, 198]

    counts = np.bincount(graph_id, minlength=G).astype(np.float32)
    recip_counts = (1.0 / np.maximum(counts, 1.0)).reshape(G, 1)
    bias_mean = bias.reshape(HEADS, HID).mean(axis=0)  # [HID]
    bias_pooled = np.tile(bias_mean, (G, 1)).astype(np.float32)  # [G, HID]
    lin1_b_rep = np.tile(lin1_b, (G, 1)).astype(np.float32)      # [G, HID]

    own_pad = ((own + 511) // 512) * 512

    in_maps = []
    for c in range(NCORES):
        tiles = per_core_tiles[c]
        # meta layout per tile: [128, 4*KCH] int32-packed:
        #   cols 0:K src gidx | K:2K dst gidx | 2K:3K seg(f32) | 3K:4K gm(f32)
        meta = np.zeros((n_tiles, 128, 4 * KCH), np.int32)
        for t, td in enumerate(tiles):
            meta[t, :, 0:KCH] = td["src"].reshape(KCH, 128).T
            meta[t, :, KCH:2 * KCH] = td["dst"].reshape(KCH, 128).T
            meta[t, :, 2 * KCH:3 * KCH] = (
                td["seg"].reshape(KCH, 128).T.astype(np.float32).view(np.int32))
            gm = td["gm"]  # [SMAX, G] -> [128, KCH] needs G<=KCH... G=16=KCH
            meta[t, :, 3 * KCH:3 * KCH + G] = gm.astype(np.float32).view(np.int32)
        # pad tiles: seg col must be SMAX-1 (f32), rest zeros
        for t in range(len(tiles), n_tiles):
            meta[t, :, 2 * KCH:3 * KCH] = (
                np.float32(SMAX - 1).view(np.int32))

        hT_own = np.zeros((NODE, own_pad), np.float32)
        hT_own[:, :own] = h[c * own:(c + 1) * own].T

        in_maps.append({
            "hT_own": hT_own,
            "meta": meta,
            "waugT": waugT,
            "identity": np.eye(128, dtype=np.float32),
            "iota_rep": np.tile(np.arange(SMAX, dtype=np.float32), (128, 1)),
            "recip_counts": recip_counts,
            "bias_pooled": bias_pooled,
            "lin1_b_rep": lin1_b_rep,
            "zT": z.T.copy(),                       # [OUT, G]
            "lin1_wT": lin1_w.T.copy(),             # [OUT, HID]
            "lin2_wT": lin2_w.T.copy(),             # [2*HID, 1]
            "lin2_b": lin2_b.reshape(1, 1).copy(),
        })
    return in_maps, n_tiles, own, own_pad


# ---------------------------------------------------------------------------
# Device program
# ---------------------------------------------------------------------------

def build_program(n_tiles, own, own_pad):
    nc = bacc.Bacc("TRN2", target_bir_lowering=False)

    # --- parameters (per-core inputs) ---
    hT_own = nc.declare_dram_parameter("hT_own", [NODE, own_pad], F32, isOutput=False)
    meta = nc.declare_dram_parameter("meta", [n_tiles, 128, 4 * KCH], I32, isOutput=False)
    waugT_p = nc.declare_dram_parameter("waugT", [NODE, WAUGW], F32, isOutput=False)
    identity_p = nc.declare_dram_parameter("identity", [128, 128], F32, isOutput=False)
    iota_p = nc.declare_dram_parameter("iota_rep", [128, SMAX], F32, isOutput=False)
    rc_p = nc.declare_dram_parameter("recip_counts", [G, 1], F32, isOutput=False)
    biasp_p = nc.declare_dram_parameter("bias_pooled", [G, HID], F32, isOutput=False)
    l1b_p = nc.declare_dram_parameter("lin1_b_rep", [G, HID], F32, isOutput=False)
    zT_p = nc.declare_dram_parameter("zT", [OUT, G], F32, isOutput=False)
    l1w_p = nc.declare_dram_parameter("lin1_wT", [OUT, HID], F32, isOutput=False)
    l2w_p = nc.declare_dram_parameter("lin2_wT", [2 * HID, 1], F32, isOutput=False)
    l2b_p = nc.declare_dram_parameter("lin2_b", [1, 1], F32, isOutput=False)
    out_p = nc.declare_dram_parameter("out", [1, G], F32, isOutput=True)
    if DEBUG_TAPS:
        dbg_feat_p = nc.declare_dram_parameter(
            "dbg_feat", [2, 1024, AUGW], F32, isOutput=True)
        dbg_er_p = nc.declare_dram_parameter(
            "dbg_er", [2, 1024, HEADS], F32, isOutput=True)
        dbg_fs_p = nc.declare_dram_parameter(
            "dbg_fs", [128, KCH, AUGW], F32, isOutput=True)
        dbg_ert_p = nc.declare_dram_parameter(
            "dbg_ert", [128, KCH, HEADS], F32, isOutput=True)
        dbg_numer_p = nc.declare_dram_parameter(
            "dbg_numer", [SMAX, XW], F32, isOutput=True)
        dbg_pool_p = nc.declare_dram_parameter(
            "dbg_pool", [G, HID], F32, isOutput=True)

    # --- internal DRAM ---
    featAug_own = nc.dram_tensor("featAug_own", [own, AUGW], F32)
    er_own = nc.dram_tensor("er_own", [own, HEADS], F32)
    featAug_full = nc.dram_tensor(
        "featAug_full", [N, AUGW], F32, addr_space="Shared")
    er_full = nc.dram_tensor("er_full", [N, HEADS], F32, addr_space="Shared")
    pooled_dram = nc.dram_tensor("pooled_dram", [G, HID], F32)
    pooled_ar = nc.dram_tensor("pooled_ar", [G, HID], F32, addr_space="Shared")

    replica_groups = [list(range(NCORES))]

    with ExitStack() as top:
        tc = top.enter_context(tile.TileContext(nc))

        # persistent constants
        const_pool = top.enter_context(tc.tile_pool(name="const", bufs=1))
        mmdt = F32R if MM_FAST else F32
        waugT = const_pool.tile([NODE, WAUGW], F32)
        identity = const_pool.tile([128, 128], F32)
        iota_rep = const_pool.tile([128, SMAX], F32)
        nc.sync.dma_start(waugT[:], waugT_p[:])
        nc.sync.dma_start(identity[:], identity_p[:])
        nc.sync.dma_start(iota_rep[:], iota_p[:])
        waugT_r = const_pool.tile([NODE, WAUGW], mmdt)
        nc.vector.tensor_copy(waugT_r[:], waugT[:])

        # =============== Phase A: node tables ===============
        n_chunks = own_pad // 512
        with ExitStack() as pa:
            sb = pa.enter_context(tc.tile_pool(name="pa_sb", bufs=3))
            ps = pa.enter_context(tc.tile_pool(name="pa_ps", bufs=2, space="PSUM"))
            ps2 = pa.enter_context(tc.tile_pool(name="pa_ps2", bufs=2, space="PSUM"))
            for ci in range(n_chunks):
                n_valid = min(512, own - ci * 512)
                if n_valid <= 0:
                    break
                ht = sb.tile([NODE, 512], F32, tag="ht")
                nc.gpsimd.dma_start(ht[:], hT_own[:, ci * 512:(ci + 1) * 512])
                ht_r = sb.tile([NODE, 512], mmdt, tag="ht_r")
                nc.vector.tensor_copy(ht_r[:], ht[:])
                fA = ps.tile([128, 512], F32, tag="fA", space="PSUM")
                fB = ps.tile([WAUGW - 128, 512], F32, tag="fB", space="PSUM")
                nc.tensor.matmul(
                    fA[:], lhsT=waugT_r[:, 0:128],
                    rhs=ht_r[:], start=True, stop=True,
                    skip_group_check=True)
                nc.tensor.matmul(
                    fB[:], lhsT=waugT_r[:, 128:WAUGW],
                    rhs=ht_r[:], start=True, stop=True,
                    skip_group_check=True)
                fA_sb = sb.tile([128, 512], F32, tag="fA_sb")
                fB_sb = sb.tile([WAUGW - 128, 512], F32, tag="fB_sb")
                nc.vector.tensor_copy(fA_sb[:], fA[:])
                nc.scalar.copy(fB_sb[:], fB[:])
                rows = sb.tile([128, 4, WAUGW], F32, tag="rows")
                for c2 in range(4):
                    tA = ps2.tile([128, 128], F32, tag="tA", space="PSUM")
                    nc.tensor.transpose(
                        tA[:], fA_sb[:, c2 * 128:(c2 + 1) * 128], identity[:])
                    tB = ps2.tile([128, WAUGW - 128], F32, tag="tB", space="PSUM")
                    bw = WAUGW - 128
                    nc.tensor.transpose(
                        tB[:], fB_sb[:, c2 * 128:(c2 + 1) * 128],
                        identity[0:bw, 0:bw])
                    nc.vector.tensor_copy(rows[:, c2, 0:128], tA[:])
                    nc.vector.tensor_copy(rows[:, c2, 128:WAUGW], tB[:])
                # write out featAug rows [n_valid, 195] and er rows [n_valid, 3]
                base = ci * 512
                nw = n_valid
                for c2 in range((nw + 127) // 128):
                    r = min(128, nw - c2 * 128)
                    nc.scalar.dma_start(
                        featAug_own[base + c2 * 128:base + c2 * 128 + r, :],
                        rows[:r, c2, 0:AUGW])
                    nc.sync.dma_start(
                        er_own[base + c2 * 128:base + c2 * 128 + r, :],
                        rows[:r, c2, AUGW:WAUGW])

        # AllGather node tables
        nc.gpsimd.collective_compute(
            "AllGather", mybir.AluOpType.bypass,
            replica_groups=replica_groups,
            ins=[featAug_own[:]], outs=[featAug_full[:]])
        nc.gpsimd.collective_compute(
            "AllGather", mybir.AluOpType.bypass,
            replica_groups=replica_groups,
            ins=[er_own[:]], outs=[er_full[:]])
        if DEBUG_TAPS:
            half = N // 2
            nc.sync.dma_start(dbg_feat_p[0], featAug_full[0:1024])
            nc.sync.dma_start(dbg_feat_p[1], featAug_full[half:half + 1024])
            nc.sync.dma_start(dbg_er_p[0], er_full[0:1024])
            nc.sync.dma_start(dbg_er_p[1], er_full[half:half + 1024])

        # =============== Phase B: edge tiles ===============
        with ExitStack() as pb:
            sbm = pb.enter_context(tc.tile_pool(name="pb_meta", bufs=3))
            sbf = pb.enter_context(tc.tile_pool(name="pb_fs", bufs=2))
            sbw = pb.enter_context(tc.tile_pool(name="pb_w", bufs=3))
            sbx = pb.enter_context(tc.tile_pool(name="pb_x", bufs=1))
            psn = pb.enter_context(tc.tile_pool(name="pb_ps", bufs=2, space="PSUM"))
            psp = pb.enter_context(tc.tile_pool(name="pb_psp", bufs=2, space="PSUM"))
            accp = pb.enter_context(tc.tile_pool(name="pb_acc", bufs=1))

            pooled_acc = accp.tile([G, HID], F32)
            nc.vector.memset(pooled_acc[:], 0.0)

            # manually double-buffered X (junk cols 195:256 zeroed once)
            xw = XW if MM_FAST else 196
            xbufs = [accp.tile([128, KCH, xw], mmdt, tag=f"xb{i}",
                               name=f"xb{i}")
                     for i in range(2)]
            zsrc = accp.tile([128, KCH, xw], F32)
            nc.vector.memset(zsrc[:], 0.0)
            for xb in xbufs:
                nc.vector.tensor_copy(xb[:], zsrc[:])

            for t in range(n_tiles):
                mt = sbm.tile([128, 4 * KCH], I32, tag="meta")
                nc.sync.dma_start(mt[:], meta[t])
                mt_f = mt[:].bitcast(F32)

                fs = sbf.tile([128, KCH, AUGW], F32, tag="fs")
                nc.gpsimd.indirect_dma_start(
                    out=fs[:], out_offset=None,
                    in_=featAug_full[:],
                    in_offset=bass.IndirectOffsetOnAxis(
                        ap=mt[:, 0:KCH], axis=0))
                ert = sbw.tile([128, KCH, HEADS], F32, tag="ert")
                nc.gpsimd.indirect_dma_start(
                    out=ert[:], out_offset=None,
                    in_=er_full[:],
                    in_offset=bass.IndirectOffsetOnAxis(
                        ap=mt[:, KCH:2 * KCH], axis=0))

                if DEBUG_TAPS and t == DBG_TILE:
                    nc.sync.dma_start(dbg_fs_p[:], fs[:])
                    nc.sync.dma_start(dbg_ert_p[:], ert[:])

                # w = exp(leaky_relu(el + er))
                w = sbw.tile([128, KCH, HEADS], F32, tag="w")
                nc.vector.tensor_add(
                    w[:], fs[:, :, FW:AUGW], ert[:])
                nc.vector.scalar_tensor_tensor(
                    out=w[:], in0=w[:], scalar=float(NEG_SLOPE), in1=w[:],
                    op0=mybir.AluOpType.mult, op1=mybir.AluOpType.max)
                nc.scalar.activation(
                    w[:], w[:], mybir.ActivationFunctionType.Exp)

                # build M (one-hot segment matrix), all chunks at once
                m_all = sbw.tile([128, KCH, SMAX], mmdt, tag="m")
                seg = mt_f[:, 2 * KCH:3 * KCH]
                nc.vector.tensor_tensor(
                    out=m_all[:],
                    in0=seg[:, :, None].to_broadcast([128, KCH, SMAX]),
                    in1=iota_rep[:, None, :].to_broadcast([128, KCH, SMAX]),
                    op=mybir.AluOpType.is_equal)

                # X = [w*feat | w | 0-pad]
                x = xbufs[t % 2]
                for k in range(HEADS):
                    nc.vector.tensor_tensor(
                        out=x[:, :, k * HID:(k + 1) * HID],
                        in0=fs[:, :, k * HID:(k + 1) * HID],
                        in1=w[:, :, k:k + 1].to_broadcast([128, KCH, HID]),
                        op=mybir.AluOpType.mult)
                nc.vector.tensor_copy(x[:, :, FW:FW + HEADS], w[:])

                # segment reduce: numer[s, :] = sum_e M[e,s] X[e,:]
                numer = psn.tile([SMAX, xw], F32, tag="numer", space="PSUM")
                for c in range(KCH):
                    nc.tensor.matmul(
                        numer[:], lhsT=m_all[:, c, :],
                        rhs=x[:, c, :],
                        start=(c == 0), stop=(c == KCH - 1),
                        skip_group_check=True)

                if DEBUG_TAPS and t == DBG_TILE:
                    numer_sb = sbw.tile([SMAX, xw], F32, tag="numer_sb")
                    nc.vector.tensor_copy(numer_sb[:], numer[:])
                    nc.sync.dma_start(dbg_numer_p[:, 0:xw], numer_sb[:])

                # alpha divide + head sum
                denom = sbw.tile([SMAX, HEADS], F32, tag="denom")
                nc.vector.tensor_scalar_max(
                    denom[:], numer[:, FW:FW + HEADS], 1e-30)
                recip = sbw.tile([SMAX, HEADS], F32, tag="recip")
                nc.vector.reciprocal(recip[:], denom[:])
                hd = sbw.tile([SMAX, HID], F32, tag="hd")
                nc.vector.tensor_scalar_mul(
                    hd[:], numer[:, 0:HID], recip[:, 0:1])
                for k in (1, 2):
                    nc.vector.scalar_tensor_tensor(
                        out=hd[:], in0=numer[:, k * HID:(k + 1) * HID],
                        scalar=recip[:, k:k + 1], in1=hd[:],
                        op0=mybir.AluOpType.mult, op1=mybir.AluOpType.add)

                # pooled partial: gm^T @ hd
                pooled_ps = psp.tile([G, HID], F32, tag="pooled", space="PSUM")
                nc.tensor.matmul(
                    pooled_ps[:], lhsT=mt_f[:, 3 * KCH:3 * KCH + G],
                    rhs=hd[:], start=True, stop=True, skip_group_check=True)
                nc.vector.tensor_add(pooled_acc[:], pooled_acc[:], pooled_ps[:])

            # =============== Finish: AllReduce + MLP ===============
            if DEBUG_TAPS:
                nc.sync.dma_start(dbg_pool_p[:], pooled_acc[:])
            nc.sync.dma_start(pooled_dram[:], pooled_acc[:])
            nc.gpsimd.collective_compute(
                "AllReduce", mybir.AluOpType.add,
                replica_groups=replica_groups,
                ins=[pooled_dram[:]], outs=[pooled_ar[:]])

            fin = pb.enter_context(tc.tile_pool(name="fin", bufs=1))
            finp = pb.enter_context(tc.tile_pool(name="finp", bufs=1, space="PSUM"))
            pooled_sb = fin.tile([G, HID], F32)
            nc.sync.dma_start(pooled_sb[:], pooled_ar[:])
            rc = fin.tile([G, 1], F32)
            nc.sync.dma_start(rc[:], rc_p[:])
            biasp = fin.tile([G, HID], F32)
            nc.sync.dma_start(biasp[:], biasp_p[:])
            l1b = fin.tile([G, HID], F32)
            nc.sync.dma_start(l1b[:], l1b_p[:])
            zT = fin.tile([OUT, G], F32)
            nc.sync.dma_start(zT[:], zT_p[:])
            l1w = fin.tile([OUT, HID], F32)
            nc.sync.dma_start(l1w[:], l1w_p[:])
            l2w = fin.tile([2 * HID, 1], F32)
            nc.sync.dma_start(l2w[:], l2w_p[:])
            l2b = fin.tile([1, 1], F32)
            nc.sync.dma_start(l2b[:], l2b_p[:])

            cat = fin.tile([G, 2 * HID], F32)
            # cat[:, :HID] = pooled/counts + bias_mean
            nc.vector.tensor_scalar_mul(
                cat[:, 0:HID], pooled_sb[:], rc[:, 0:1])
            nc.vector.tensor_add(cat[:, 0:HID], cat[:, 0:HID], biasp[:])
            # cat[:, HID:] = z @ lin1_w.T + lin1_b
            z1_ps = finp.tile([G, HID], F32, tag="z1", space="PSUM")
            nc.tensor.matmul(z1_ps[:], lhsT=zT[:], rhs=l1w[:],
                             start=True, stop=True, skip_group_check=True)
            nc.vector.tensor_add(cat[:, HID:2 * HID], z1_ps[:], l1b[:])
            # score = sigmoid(cat @ lin2_w.T + b)  -> computed as [1, G]
            catT_ps = finp.tile([2 * HID, G], F32, tag="catT", space="PSUM")
            nc.tensor.transpose(catT_ps[:], cat[:], identity[0:G, 0:G])
            catT = fin.tile([2 * HID, G], F32)
            nc.vector.tensor_copy(catT[:], catT_ps[:])
            score_ps = finp.tile([1, G], F32, tag="score", space="PSUM")
            nc.tensor.matmul(score_ps[:], lhsT=l2w[:], rhs=catT[:],
                             start=True, stop=True, skip_group_check=True)
            out_sb = fin.tile([1, G], F32)
            nc.scalar.activation(
                out_sb[:], score_ps[:], mybir.ActivationFunctionType.Sigmoid,
                bias=l2b[:, 0:1])
            nc.sync.dma_start(out_p[:], out_sb[:])

    nc.finalize()
    return nc


# ---------------------------------------------------------------------------
# Entry point
# ---------------------------------------------------------------------------

def kernel(**inputs) -> np.ndarray:
    in_maps, n_tiles, own, own_pad = _host_prepare(inputs)
    nc = build_program(n_tiles, own, own_pad)
    res = run_bass_kernel_spmd(nc, in_maps, list(range(NCORES)))
    out = res.results[0]["out"]  # [1, G]
    return np.asarray(out, np.float32).reshape(G, 1)

